# revision 5
# baseline (speedup 1.0000x reference)
"""Trainium2 Bass kernel for nn_Loss_Synonymy.

reference:
    diff = S1 - S2                       # [B, 256]
    d    = sqrt(sum(diff^2, axis=-1))    # [B]
    t    = tanh(d)
    err  = where(score >= 0.8, relu(1 - t), relu(1 + t))
    out  = sum(err) / B

Since tanh(d) in [0, 1) for d >= 0, relu(1 -+ tanh(d)) = 1 -+ tanh(d), so
err = 1 + sgn * tanh(d) with sgn = -1 (score >= 0.8) else +1, and
sum(err) = B + sum(sgn * tanh(d)).  The kernel only accumulates
sgn * tanh(d); the host adds B and divides.

Data-parallel over 8 NeuronCores, 32768 rows each.  Per-core layout:
partition p owns rows [p*256, (p+1)*256) of the shard, so the score
vector is ONE contiguous [128, 256] HWDGE load and the per-row sums
land as [128, 256] aligned with it.  s1/s2 are stacked host-side into
x[2, BL, D] so each tile is a single 4 MiB dma_start (2 x 16 KiB
contiguous per partition).  Tile t covers row-offsets [off, off+J) of
every partition's 256-row block:

    DMA  (sync HWDGE): X[128, 2*J*256] <- x[:, p*256+off : .. +J, :]
    DVE : diff = a - b           (in place, second half of X)
    ACT : sq   = Square(diff)    (in place)
    DVE : sumsq[:, off:off+J] = reduce_add(sq.view(128, J, 256), axis=X)

15 tiles of J=16 then 4 taper tiles of J=4 (shorter drain after the
last DMA).  Sqrt/Tanh activation tables are preloaded during the DMA
ramp.  Epilogue (single pass over [128, 256]):

    dist = Sqrt(sumsq); th = Tanh(dist)                  (ACT)
    sgn2 = (score >= 0.8) * -2                           (DVE)
    err  = (sgn2 + 1) * th, accum -> part[128, 1]        (DVE)

Host: out = (B + sum(partials)) / B.
"""

import numpy as np

import concourse.bass as bass
import concourse.tile as tile
from concourse import bacc, mybir
from concourse.bass_utils import run_bass_kernel_spmd

F32 = mybir.dt.float32
AF = mybir.ActivationFunctionType
ALU = mybir.AluOpType

B = 262144
D = 256
NCORES = 8
BL = B // NCORES          # 32768 rows per core
RPP = BL // 128           # 256 rows per partition
THRESH = 0.8

# (J, count): per-partition row-chunks per tile; sum(J*count) == RPP
TILING = [(16, 15), (4, 4)]
BUFS_BIG = 4
BUFS_SMALL = 4

_NC_CACHE = {}


def _build_nc():
    nc = bacc.Bacc(
        "TRN2", target_bir_lowering=False, debug=False, num_devices=NCORES
    )

    x = nc.dram_tensor("x", [2, BL, D], F32, kind="ExternalInput").ap()
    score = nc.dram_tensor("score", [BL], F32, kind="ExternalInput").ap()
    partial = nc.dram_tensor("partial", [128, 1], F32, kind="ExternalOutput").ap()

    # [128, 2, 256, 256]: partition p / source s / row-in-block c / feature d
    x_r = x.rearrange("s (p c) d -> p s c d", p=128, c=RPP)
    score_r = score.rearrange("(p c) -> p c", p=128, c=RPP)

    with tile.TileContext(nc) as tc:
        with (
            tc.tile_pool(name="big", bufs=BUFS_BIG) as p_big,
            tc.tile_pool(name="small", bufs=BUFS_SMALL) as p_small,
            tc.tile_pool(name="persist", bufs=1) as p_per,
        ):
            sumsq = p_per.tile([128, RPP], F32, tag="sumsq")
            score_sb = p_per.tile([128, RPP], F32, tag="score_sb")
            part_sb = p_per.tile([128, 1], F32, tag="part_sb")

            # Contiguous score load on the HWDGE ring (1 KiB per partition).
            nc.sync.dma_start(score_sb[:], score_r)

            # (score >= 0.8) * -2  ->  {-2, 0}; hidden under the DMA ramp.
            sgn2 = p_per.tile([128, RPP], F32, tag="sgn2")
            nc.vector.tensor_scalar(
                sgn2[:], score_sb[:], THRESH, -2.0, ALU.is_ge, ALU.mult
            )

            # Software-pipelined by one stage: reduce_t is emitted after
            # sub_{t+1}/square_{t+1}, so the in-order DVE never stalls
            # waiting on ACT's square of the same tile.
            pending = None  # (b_half, off, J) awaiting its reduce

            def emit_reduce(p):
                b, off, J = p
                nc.vector.tensor_reduce(
                    sumsq[:, off : off + J],
                    b.rearrange("p (j d) -> p j d", d=D),
                    axis=mybir.AxisListType.X,
                    op=ALU.add,
                )

            off = 0
            for J, count in TILING:
                FREE = J * D
                pool = p_big if J == TILING[0][0] else p_small
                for _ in range(count):
                    X = pool.tile([128, 2 * FREE], F32, tag=f"x{J}")
                    nc.sync.dma_start(
                        X[:].rearrange("p (s j d) -> p s j d", s=2, d=D),
                        x_r[:, :, off : off + J, :],
                    )
                    a = X[:, 0:FREE]
                    b = X[:, FREE : 2 * FREE]
                    nc.vector.tensor_sub(b, a, b)
                    nc.scalar.activation(b, b, AF.Square)
                    if pending is not None:
                        emit_reduce(pending)
                    pending = (b, off, J)
                    off += J
            emit_reduce(pending)

            # Epilogue: part = sum_p sgn * tanh(d), d = sumsq * rsqrt(sumsq).
            # rsqrt instead of Sqrt in the hope of staying in Tanh's ACT
            # table set; tanh(d) saturates at these distances (~16), so
            # rsqrt accuracy is irrelevant.
            rs = p_per.tile([128, RPP], F32, tag="rs")
            nc.scalar.activation(rs[:], sumsq[:], AF.Abs_reciprocal_sqrt)
            dist = p_per.tile([128, RPP], F32, tag="dist")
            nc.vector.tensor_mul(dist[:], sumsq[:], rs[:])
            th = p_per.tile([128, RPP], F32, tag="th")
            nc.scalar.activation(th[:], dist[:], AF.Tanh)
            # (sgn2 + 1) * th -> +-tanh, accumulated per partition
            err = p_per.tile([128, RPP], F32, tag="err")
            nc.vector.scalar_tensor_tensor(
                err[:], sgn2[:], 1.0, th[:], ALU.add, ALU.mult,
                accum_out=part_sb[:],
            )

            nc.sync.dma_start(partial, part_sb[:])

    nc.compile()
    return nc


def _get_nc():
    if "nc" not in _NC_CACHE:
        _NC_CACHE["nc"] = _build_nc()
    return _NC_CACHE["nc"]


def make_in_maps(S1_out, S2_out, synonymy_score):
    in_maps = []
    for c in range(NCORES):
        lo, hi = c * BL, (c + 1) * BL
        x = np.empty((2, BL, D), dtype=np.float32)
        x[0] = S1_out[lo:hi]
        x[1] = S2_out[lo:hi]
        in_maps.append(
            {
                "x": x,
                "score": np.ascontiguousarray(
                    synonymy_score[lo:hi], dtype=np.float32
                ),
            }
        )
    return in_maps


def combine(results):
    total = np.float64(B)
    for r in results:
        total += r["partial"].astype(np.float64).sum()
    return np.asarray(total / B, dtype=np.float32)


def run(S1_out, S2_out, synonymy_score, trace=False, **trace_kwargs):
    nc = _get_nc()
    in_maps = make_in_maps(S1_out, S2_out, synonymy_score)
    res = run_bass_kernel_spmd(
        nc, in_maps, list(range(NCORES)), trace=trace, **trace_kwargs
    )
    return combine(res.results), res


def kernel(S1_out, S2_out, synonymy_score):
    out, _ = run(S1_out, S2_out, synonymy_score)
    return out
